# revision 1
# baseline (speedup 1.0000x reference)
"""Trainium2 Bass kernel for batched GCN (2x GCNConv + circular Conv1d).

Math per graph (N=64 nodes, S=96 feats, H=512 hidden, E=512 edges):
    deg[d]   = #edges with dst=d (incl. self loop)
    A        = Dinv @ (M0 + I).T @ Dinv,  Dinv = diag(1/sqrt(deg))
    h1       = relu(A @ (x.T @ W1.T) + b1)
    h2       = A @ (h1 @ W2.T) + b2
    y        = circular_conv1d(h2, conv_w).T          # [96, 512]

Device strategy (per core: 64 graphs, processed as 32 pairs of 2 graphs
occupying partition halves 0-63 / 64-127):
  - M0.T built per graph with one-hot matmuls: onehots from a single DVE
    is_equal against an iota table (broadcast APs), then 4 K=128 matmuls
    + identity matmul accumulate counts in PSUM.
  - Row-scale M0.T by dinv (DVE), block-diagonal pair tile, one PE
    transpose -> block-diag Ms = (M_aug @ Dinv) for the pair.
  - agg1 computed transposed (H on partitions) to feed gcn2 without extra
    transposes; dinv folded into z1/z2 copies; bf16 matmuls (fp32 PSUM).
  - conv done as 3 shifted-tap matmuls per graph on a duplicated [h2|h2]
    tile; output lands [l, o]-major in PSUM, copied once, DMA'd out.
"""

import numpy as np
import ml_dtypes

import concourse.bacc as bacc
import concourse.mybir as mybir
import concourse.tile as tile
from concourse.bass_utils import run_bass_kernel_spmd

BF16 = mybir.dt.bfloat16
FP32 = mybir.dt.float32
I32 = mybir.dt.int32
AF = mybir.ActivationFunctionType

N_CORES = 8
B, S, N, H, E = 512, 96, 64, 512, 512
G = B // N_CORES          # graphs per core
NPAIR = G // 2


def build_gcn_kernel(tc, outs, ins, g_per_core=G, has_b1=False, has_b2=False):
    """Emit the kernel into TileContext tc. outs/ins are dicts of DRAM APs."""
    nc = tc.nc
    g = g_per_core
    npair = g // 2

    x_d = ins["x"]          # [g, 96, 64] f32
    ei_d = ins["ei"]        # [2g, 512] i32   (row = 2*graph + (0:src,1:dst))
    w1t_d = ins["w1t"]      # [96, 512] bf16
    w2t_d = ins["w2t"]      # [128, 384] bf16 (f=(c,s))
    cwd_d = ins["cwd"]      # [128, 1536] bf16 (rows 0-63 = [i,(k,o)], dup)
    iota_d = ins["iota"]    # [128, 1024] bf16 (f%64)
    i64d_d = ins["i64d"]    # [128, 64] bf16 (I64 stacked twice)
    id128_d = ins["id128"]  # [128, 128] bf16
    y_d = outs["y"]         # [g, 96, 512] f32

    from contextlib import ExitStack
    ctx = ExitStack()
    const = ctx.enter_context(tc.tile_pool(name="const", bufs=1))
    sb = ctx.enter_context(tc.tile_pool(name="sb", bufs=6))
    psa = ctx.enter_context(tc.tile_pool(name="psa", bufs=3, space="PSUM"))
    ps = ctx.enter_context(tc.tile_pool(name="ps", bufs=2, space="PSUM"))
    ps1 = ctx.enter_context(tc.tile_pool(name="ps1", bufs=1, space="PSUM"))
    psy = ctx.enter_context(tc.tile_pool(name="psy", bufs=2, space="PSUM"))

    # ---- constants ----
    w1t = const.tile([96, 512], BF16)
    nc.sync.dma_start(out=w1t[:], in_=w1t_d[:])
    w2t = const.tile([128, 384], BF16)
    nc.sync.dma_start(out=w2t[:], in_=w2t_d[:])
    cwd = const.tile([128, 1536], BF16)
    nc.sync.dma_start(out=cwd[:], in_=cwd_d[:])
    iota = const.tile([128, 1024], BF16)
    nc.sync.dma_start(out=iota[:], in_=iota_d[:])
    i64d = const.tile([128, 64], BF16)
    nc.sync.dma_start(out=i64d[:], in_=i64d_d[:])
    id128 = const.tile([128, 128], BF16)
    nc.sync.dma_start(out=id128[:], in_=id128_d[:])
    id128f = const.tile([128, 128], FP32)
    nc.vector.tensor_copy(out=id128f[:], in_=id128[:])
    if has_b1:
        b1c = const.tile([128, 4], FP32)
        nc.sync.dma_start(out=b1c[:], in_=ins["b1c"][:])
    if has_b2:
        b2d = const.tile([128, 192], BF16)
        nc.sync.dma_start(out=b2d[:], in_=ins["b2d"][:])

    # ---- x: load + cast to bf16, laid out [s, (g, n)] ----
    xf = const.tile([96, 64 * g], FP32)
    nc.sync.dma_start(out=xf[:].rearrange("s (g n) -> s g n", g=g),
                      in_=x_d[:].rearrange("g s n -> s g n"))
    xbf = const.tile([96, 64 * g], BF16)
    nc.vector.tensor_copy(out=xbf[:], in_=xf[:])

    # ---- edges: load, cast, transpose to [epos, (c, gt)] ----
    ei = const.tile([2 * g, 512], I32)
    nc.sync.dma_start(out=ei[:], in_=ei_d[:])
    eibf = const.tile([2 * g, 512], BF16)
    nc.vector.tensor_copy(out=eibf[:], in_=ei[:])
    et = const.tile([128, 4 * 2 * g], BF16)   # f = (c, gt)
    for c in range(4):
        etp = ps.tile([128, 128], BF16, tag="z1")
        # in_ is [2g, 128] -> out = in_.T = [128, 2g]
        nc.tensor.transpose(
            out=etp[:, 0:2 * g], in_=eibf[:, c * 128:(c + 1) * 128],
            identity=id128[0:2 * g, 0:2 * g],
        )
        nc.scalar.activation(
            out=et[:, c * 2 * g:(c + 1) * 2 * g], in_=etp[:, 0:2 * g],
            func=AF.Copy,
        )

    for pr in range(npair):
        # ---- z1 = x^T W1^T for the pair (independent of A-chain) ----
        z1_ps = ps.tile([128, 512], FP32, tag="z1")
        nc.tensor.matmul(z1_ps[:], xbf[:, 128 * pr:128 * (pr + 1)], w1t[:],
                         start=True, stop=True)

        # ---- one-hots: oh[p, (c, j, v)] = (et[p, (c, 4pr+j)] == v) ----
        e_sl = et[:].rearrange("p (c gt) -> p c gt", c=4)
        e_sl = e_sl[:, :, 4 * pr:4 * pr + 4]
        e_sl = e_sl.rearrange("p c (j u) -> p c j u", u=1)
        e_bc = e_sl.to_broadcast([128, 4, 4, 64])
        erep = sb.tile([128, 1024], BF16, tag="erep")
        nc.gpsimd.tensor_copy(
            out=erep[:].rearrange("p (c j v) -> p c j v", c=4, j=4),
            in_=e_bc)
        oh = sb.tile([128, 1024], BF16, tag="oh")
        nc.vector.tensor_tensor(
            out=oh[:], in0=erep[:], in1=iota[:],
            op=mybir.AluOpType.is_equal,
        )

        # ---- M_aug^T (counts + I) per graph into pair psum [128, 64] ----
        mps = psa.tile([128, 384], FP32, tag="mzz")
        maug = mps[:, 0:64]
        for gl in range(2):
            po = 64 * gl
            out_sl = maug[po:po + 64, :]  # noqa
            tp = None if gl == 0 else (0, 64)
            for c in range(4):
                base = c * 256
                lhsT = oh[:, base + (2 * gl + 1) * 64: base + (2 * gl + 2) * 64]
                rhs = oh[:, base + (2 * gl) * 64: base + (2 * gl + 1) * 64]
                nc.tensor.matmul(out_sl, lhsT, rhs, start=(c == 0),
                                 stop=False, tile_position=tp)
            nc.tensor.matmul(
                out_sl, i64d[po:po + 64, :], i64d[po:po + 64, :],
                start=False, stop=True,
                tile_position=None if gl == 0 else (64, 64),
            )

        # ---- deg -> dinv ----
        deg = sb.tile([128, 1], FP32, tag="deg")
        nc.vector.tensor_reduce(out=deg[:], in_=maug[:, :],
                                axis=mybir.AxisListType.X,
                                op=mybir.AluOpType.add)
        sq = sb.tile([128, 1], FP32, tag="sq")
        nc.scalar.activation(out=sq[:], in_=deg[:], func=AF.Sqrt)
        dinv = sb.tile([128, 1], FP32, tag="dinv")
        nc.vector.reciprocal(out=dinv[:], in_=sq[:])

        # ---- MsT block-diag -> transpose -> Ms block-diag (bf16) ----
        msb = sb.tile([128, 128], FP32, tag="msb")
        nc.gpsimd.memset(msb[:], 0)
        nc.vector.tensor_scalar(
            out=msb[0:64, 0:64], in0=maug[0:64, :], scalar1=dinv[0:64, :],
            scalar2=None, op0=mybir.AluOpType.mult)
        nc.scalar.activation(
            out=msb[64:128, 64:128], in_=maug[64:128, :], func=AF.Copy,
            scale=dinv[64:128, :])
        mst_ps = mps[:, 64:192]
        nc.tensor.transpose(out=mst_ps, in_=msb[:], identity=id128f[:])
        msbd = sb.tile([128, 128], BF16, tag="msbd")
        nc.scalar.activation(out=msbd[:], in_=mst_ps, func=AF.Copy)

        # ---- z1s = dinv * z1 ----
        z1s = sb.tile([128, 512], BF16, tag="z1s")
        nc.vector.tensor_scalar(out=z1s[:], in0=z1_ps[:], scalar1=dinv[:, :],
                                scalar2=None, op0=mybir.AluOpType.mult)

        # ---- agg1T: [128 (h in chunk), (c, g, n)] ----
        a1t_ps = ps1.tile([128, 512], FP32, tag="a1t")
        for c in range(4):
            nc.tensor.matmul(a1t_ps[:, 128 * c:128 * (c + 1)],
                             z1s[:, 128 * c:128 * (c + 1)], msbd[:],
                             start=True, stop=True)
        h1t = sb.tile([128, 512], BF16, tag="h1t")
        if has_b1:
            for c in range(4):
                nc.scalar.activation(
                    out=h1t[:, 128 * c:128 * (c + 1)],
                    in_=a1t_ps[:, 128 * c:128 * (c + 1)],
                    func=AF.Relu, bias=b1c[:, c:c + 1])
        else:
            nc.vector.tensor_scalar_max(h1t[:], a1t_ps[:], 0.0)

        # ---- z2 = h1 W2^T: [128 (g,n), 96 (s)] ----
        z2_ps = mps[:, 192:288]
        for c in range(4):
            nc.tensor.matmul(z2_ps, h1t[:, 128 * c:128 * (c + 1)],
                             w2t[:, 96 * c:96 * (c + 1)],
                             start=(c == 0), stop=(c == 3))
        z2s = sb.tile([128, 96], BF16, tag="z2s")
        nc.scalar.activation(out=z2s[:], in_=z2_ps, func=AF.Copy,
                             scale=dinv[:, :])

        # ---- agg2: [128 (g,n), 96 (l)] ----
        a2_ps = mps[:, 288:384]
        nc.tensor.matmul(a2_ps, msbd[:], z2s[:], start=True, stop=True)

        # ---- h2 duplicated [h2|h2] (+b2); a2 is already fully aggregated ----
        hp = sb.tile([128, 192], BF16, tag="hp")
        nc.vector.tensor_copy(out=hp[:, 0:96], in_=a2_ps)
        nc.scalar.activation(out=hp[:, 96:192], in_=a2_ps, func=AF.Copy)
        if has_b2:
            hpb = sb.tile([128, 192], BF16, tag="hpb")
            nc.vector.tensor_tensor(out=hpb[:], in0=hp[:], in1=b2d[:],
                                    op=mybir.AluOpType.add)
            hp = hpb

        # ---- conv: per graph 3 shifted-tap matmuls -> [96 (l), 512 (o)] ----
        for gl in range(2):
            po = 64 * gl
            y_ps = psy.tile([96, 512], FP32, tag="y")
            for k in range(3):
                tap = (95, 0, 1)[k]
                nc.tensor.matmul(
                    y_ps[:],
                    hp[po:po + 64, tap:tap + 96],
                    cwd[po:po + 64, 512 * k:512 * (k + 1)],
                    start=(k == 0), stop=(k == 2))
            ysb = sb.tile([96, 512], FP32, tag="ysb")
            if gl == 0:
                nc.vector.tensor_copy(out=ysb[:], in_=y_ps[:])
            else:
                nc.scalar.activation(out=ysb[:], in_=y_ps[:], func=AF.Copy)
            nc.sync.dma_start(out=y_d[2 * pr + gl], in_=ysb[:])

    ctx.close()


# ---------------- host side ----------------

def _prep_consts(W1, b1, W2, b2, conv_w):
    bf = ml_dtypes.bfloat16
    w1t = np.ascontiguousarray(W1.T).astype(bf)                    # [96, 512]
    w2t = np.ascontiguousarray(
        W2.T.reshape(4, 128, 96).transpose(1, 0, 2).reshape(128, 384)
    ).astype(bf)
    base = np.ascontiguousarray(conv_w.transpose(1, 2, 0)).reshape(64, 1536)
    cwd = np.concatenate([base, base], axis=0).astype(bf)          # [128, 1536]
    iota = np.broadcast_to((np.arange(1024) % 64).astype(bf), (128, 1024))
    iota = np.ascontiguousarray(iota)
    i64d = np.concatenate([np.eye(64), np.eye(64)], axis=0).astype(bf)
    id128 = np.eye(128).astype(bf)
    consts = dict(w1t=w1t, w2t=w2t, cwd=cwd, iota=iota, i64d=i64d,
                  id128=id128)
    has_b1 = bool(np.any(b1))
    has_b2 = bool(np.any(b2))
    if has_b1:
        consts["b1c"] = np.ascontiguousarray(
            b1.reshape(4, 128).T).astype(np.float32)
    if has_b2:
        b2d = np.ascontiguousarray(
            np.broadcast_to(np.tile(b2, 2).astype(bf), (128, 192)))
        consts["b2d"] = b2d
    return consts, has_b1, has_b2


_NC_CACHE = {}


def _get_nc(g_per_core, has_b1, has_b2):
    key = (g_per_core, has_b1, has_b2)
    if key in _NC_CACHE:
        return _NC_CACHE[key]
    nc = bacc.Bacc("TRN2", target_bir_lowering=False, debug=False)
    ins = {
        "x": nc.dram_tensor("x", [g_per_core, 96, 64], FP32,
                            kind="ExternalInput").ap(),
        "ei": nc.dram_tensor("ei", [2 * g_per_core, 512], I32,
                             kind="ExternalInput").ap(),
        "w1t": nc.dram_tensor("w1t", [96, 512], BF16,
                              kind="ExternalInput").ap(),
        "w2t": nc.dram_tensor("w2t", [128, 384], BF16,
                              kind="ExternalInput").ap(),
        "cwd": nc.dram_tensor("cwd", [128, 1536], BF16,
                              kind="ExternalInput").ap(),
        "iota": nc.dram_tensor("iota", [128, 1024], BF16,
                               kind="ExternalInput").ap(),
        "i64d": nc.dram_tensor("i64d", [128, 64], BF16,
                               kind="ExternalInput").ap(),
        "id128": nc.dram_tensor("id128", [128, 128], BF16,
                                kind="ExternalInput").ap(),
    }
    if has_b1:
        ins["b1c"] = nc.dram_tensor("b1c", [128, 4], FP32,
                                    kind="ExternalInput").ap()
    if has_b2:
        ins["b2d"] = nc.dram_tensor("b2d", [128, 192], BF16,
                                    kind="ExternalInput").ap()
    outs = {
        "y": nc.dram_tensor("y", [g_per_core, 96, 512], FP32,
                            kind="ExternalOutput").ap(),
    }
    with tile.TileContext(nc) as tc:
        build_gcn_kernel(tc, outs, ins, g_per_core, has_b1, has_b2)
    nc.compile()
    _NC_CACHE[key] = nc
    return nc


def kernel(x, edge_index, W1, b1, W2, b2, conv_w, _trace=False):
    x = np.asarray(x)
    edge_index = np.asarray(edge_index)
    consts, has_b1, has_b2 = _prep_consts(
        np.asarray(W1), np.asarray(b1), np.asarray(W2), np.asarray(b2),
        np.asarray(conv_w))
    nc = _get_nc(G, has_b1, has_b2)

    bfcast = {k: v for k, v in consts.items()}
    in_maps = []
    for c in range(N_CORES):
        sl = slice(c * G, (c + 1) * G)
        m = dict(bfcast)
        m["x"] = np.ascontiguousarray(x[sl]).astype(np.float32)
        m["ei"] = np.ascontiguousarray(
            edge_index[sl].reshape(2 * G, 512)).astype(np.int32)
        in_maps.append(m)

    res = run_bass_kernel_spmd(nc, in_maps, core_ids=list(range(N_CORES)),
                               trace=_trace)
    y = np.concatenate([res.results[c]["y"] for c in range(N_CORES)], axis=0)
    if _trace:
        kernel.last_results = res
    return y



# revision 43
# speedup vs baseline: 1.0129x; 1.0129x over previous
"""Trainium2 Bass kernel for batched GCN (2x GCNConv + circular Conv1d).

Math per graph (N=64 nodes, S=96 feats, H=512 hidden, E=512 edges):
    C[d, s]  = #edges s->d  (+ I for self-loops)
    deg      = row sums of C;  dinv = 1/sqrt(deg)
    A~       = Dinv C^T Dinv        (= A^T, both dinv folded in)
    u        = X^T A~               ( = (A X)^T,  X = x.T [n, s])
    a1t      = W1 u                 (h on partitions, = (A X W1^T)^T)
    h1t      = relu(a1t)
    z2       = h1 W2^T              (via 4 h-chunk accumulation)
    h2       = A z2   (written shifted+duplicated into P for the conv)
    y        = circular_conv1d(h2, conv_w), emitted transposed [o, (g,l)]

Device strategy per core (64 graphs = 32 pairs; pair occupies partition
halves 0-63 / 64-127):
  - host pre-offsets pair-odd graphs' edge ids by +64, so a single
    is_equal against a 0..127 iota yields block one-hots and C/deg build
    as block-diagonal [128,128] matmuls (4+1 mms, deg 5 free-1 mms)
  - erep broadcast on Pool, is_equal on DVE (2x mode)
  - msb^T via the DMA transpose XBAR (PE transpose in a shared psum bank
    faults the device; so does mixing (0,64)/(64,64) tile_positions
    across matmul groups, hence g1 h2-operands are copied to parts 0:64)
  - conv: weight-stationary o-chunk matmuls, pair graphs in the free dim;
    the shifted conv input comes from matmuls reading a wrap-padded z2w
  - psum pools split by lifetime phase so many pairs pipeline
  - y lands [o_chunk, (g, l)] -> bf16 SBUF -> DMA to a transposed DRAM
    layout; host does the final transpose + f32 cast (free)
"""

import numpy as np
import ml_dtypes

import concourse.bacc as bacc
import concourse.mybir as mybir
import concourse.tile as tile
from concourse.bass_utils import run_bass_kernel_spmd

BF16 = mybir.dt.bfloat16
FP32 = mybir.dt.float32
AF = mybir.ActivationFunctionType
OP = mybir.AluOpType

N_CORES = 8
B, S, N, H, E = 512, 96, 64, 512, 512
G = B // N_CORES          # graphs per core (64)
NPAIR = G // 2            # 32


def build_gcn_kernel(tc, outs, ins, has_b1=False, has_b2=False):
    nc = tc.nc
    g = G

    xt_d = ins["xt"]        # [128, 32*96] bf16  (pair-major node rows)
    et_d = ins["et"]        # [128, 512] bf16    (c, g, side; odd graph +64)
    cst_d = ins["cst"]      # [128, Wc] bf16 packed consts
    y_d = outs["y"]         # [4, 128, 32, 192] bf16

    from contextlib import ExitStack
    ctx = ExitStack()
    const = ctx.enter_context(tc.tile_pool(name="const", bufs=1))
    sb_oh = ctx.enter_context(tc.tile_pool(name="sboh", bufs=3))
    sb = ctx.enter_context(tc.tile_pool(name="sb", bufs=3))
    sb_y = ctx.enter_context(tc.tile_pool(name="sby", bufs=3))
    # psum banks (8x2KB): psS [C128|deg|u] 1x2, psP [z2|P] 1x2, psA1 1x2,
    # psY 2 tags x 1 buf
    psS = ctx.enter_context(tc.tile_pool(name="psS", bufs=2, space="PSUM"))
    psP = ctx.enter_context(tc.tile_pool(name="psP", bufs=2, space="PSUM"))
    psA1 = ctx.enter_context(tc.tile_pool(name="psA1", bufs=2, space="PSUM"))
    psY = ctx.enter_context(tc.tile_pool(name="psY", bufs=1, space="PSUM"))

    # ---- packed consts [128, *] ----
    W_IOTA, W_ID, W_ONE, W_CWAB, W_W2T, W_W1T, W_CWC = (
        1024, 128, 1, 512, 384, 512, 512)
    Wc = W_IOTA + W_ID + W_ONE + W_CWAB + W_W2T + W_W1T + W_CWC
    cst = const.tile([128, Wc], BF16)
    nc.sync.dma_start(out=cst[:], in_=cst_d[:])
    o = 0
    iota = cst[:, o:o + W_IOTA]; o += W_IOTA
    id128 = cst[:, o:o + W_ID]; o += W_ID
    ones = cst[:, o:o + W_ONE]; o += W_ONE
    cwAB = cst[:, o:o + W_CWAB]; o += W_CWAB
    w2t = cst[:, o:o + W_W2T]; o += W_W2T
    w1t = cst[0:96, o:o + W_W1T]; o += W_W1T
    cwC = cst[0:64, o:o + W_CWC]; o += W_CWC

    if has_b1:
        b1c = const.tile([128, 4], FP32)
        nc.sync.dma_start(out=b1c[:], in_=ins["b1c"][:])
    if has_b2:
        b2r = const.tile([128, 196], BF16)
        nc.sync.dma_start(out=b2r[:], in_=ins["b2r"][:])

    xt = const.tile([128, NPAIR * 96], BF16)
    nc.sync.dma_start(out=xt[:], in_=xt_d[:])
    et = const.tile([128, 512], BF16)
    nc.sync.dma_start(out=et[:], in_=et_d[:])

    # persistent block-diag msb tiles (zero borders preserved), ping-pong
    msbs = []
    for i in range(2):
        t = const.tile([128, 128], BF16, tag=f"msb{i}")
        nc.gpsimd.memset(t[:], 0)
        msbs.append(t)

    et_r = et[:].rearrange("p (c g s) -> p c g s", c=4, g=g)

    for pr in range(NPAIR):
        # ---- one-hots: oh[p, (c, side, v128)], odd graph offset by +64 ----
        e_sl = et_r[:, :, 2 * pr:2 * pr + 2, :]
        e_bc = e_sl.rearrange("p c g (s u) -> p c s g u", u=1) \
                   .to_broadcast([128, 4, 2, 2, 64])
        erep = sb_oh.tile([128, 1024], BF16, tag="erep")
        nc.gpsimd.tensor_copy(
            out=erep[:].rearrange("p (c s g v) -> p c s g v", c=4, s=2, g=2),
            in_=e_bc)
        oh = sb_oh.tile([128, 1024], BF16, tag="oh")
        nc.vector.tensor_tensor(out=oh[:], in0=erep[:], in1=iota,
                                op=OP.is_equal)

        def ohb(c, side):
            base = c * 256 + side * 128
            return oh[:, base:base + 128]

        # ---- block-diag C (+I) and deg ----
        Sc = psS.tile([128, 257], FP32, tag="S")
        CP = Sc[:, 0:128]
        degP = Sc[:, 128:129]
        uP = Sc[0:96, 129:257]
        for c in range(4):
            nc.tensor.matmul(CP, ohb(c, 1), ohb(c, 0),
                             start=(c == 0), stop=False)
        nc.tensor.matmul(CP, id128, id128, start=False, stop=True)
        for c in range(4):
            nc.tensor.matmul(degP, ohb(c, 1), ones[:],
                             start=(c == 0), stop=False)
        nc.tensor.matmul(degP, id128, ones[:], start=False, stop=True)

        # ---- dinv = 1/sqrt(deg) ----
        sq = sb.tile([128, 1], FP32, tag="sq")
        nc.scalar.activation(out=sq[:], in_=degP, func=AF.Sqrt)
        dinv = sb.tile([128, 1], FP32, tag="dinv")
        nc.vector.reciprocal(out=dinv[:], in_=sq[:])

        # ---- msb = rowscale(diag blocks of C, dinv) ----
        msb = msbs[pr % 2]
        nc.scalar.activation(out=msb[0:64, 0:64], in_=CP[0:64, 0:64],
                             func=AF.Copy, scale=dinv[0:64, :])
        nc.scalar.activation(out=msb[64:128, 64:128], in_=CP[64:128, 64:128],
                             func=AF.Copy, scale=dinv[64:128, :])

        # ---- A~ = rowscale(msb^T, dinv); transpose via DMA XBAR ----
        mstS = sb.tile([128, 128], BF16, tag="mstS")
        nc.sync.dma_start(out=mstS[:], in_=msb[:], transpose=True)
        atil = sb.tile([128, 128], BF16, tag="atil")
        nc.vector.tensor_scalar(out=atil[:], in0=mstS[:], scalar1=dinv[:, :],
                                scalar2=None, op0=OP.mult)

        # ---- u = X^T A~  [96, 128] ----
        xts = xt[:, 96 * pr:96 * (pr + 1)]
        nc.tensor.matmul(uP, xts, atil[:], start=True, stop=True)
        u = sb.tile([96, 128], BF16, tag="u_sb")
        nc.scalar.activation(out=u[:], in_=uP, func=AF.Copy)
        if pr == 0 and "dbg" in outs:
            nc.sync.dma_start(out=outs["dbg"][0:96, 0:128], in_=u[:])
            nc.sync.dma_start(out=outs["dbg"][0:128, 128:256], in_=atil[:])
            dgs = sb.tile([128, 2], FP32, tag="dgs")
            nc.vector.tensor_copy(out=dgs[:, 0:1], in_=degP)
            nc.vector.tensor_copy(out=dgs[:, 1:2], in_=dinv[:])
            nc.sync.dma_start(out=outs["dbgf"][:], in_=dgs[:])
            nc.sync.dma_start(out=outs["dbg"][0:128, 256:1280], in_=oh[:])

        # ---- a1t = W1 u  [128 (h), (c, m)] ----
        a1tP = psA1.tile([128, 512], FP32, tag="a1t")
        for c in range(4):
            nc.tensor.matmul(a1tP[:, 128 * c:128 * (c + 1)],
                             w1t[:, 128 * c:128 * (c + 1)], u[:],
                             start=True, stop=True)
        h1t = sb.tile([128, 512], BF16, tag="h1t")
        if has_b1:
            for c in range(4):
                nc.scalar.activation(
                    out=h1t[:, 128 * c:128 * (c + 1)],
                    in_=a1tP[:, 128 * c:128 * (c + 1)],
                    func=AF.Relu, bias=b1c[:, c:c + 1])
        else:
            nc.scalar.activation(out=h1t[:], in_=a1tP[:], func=AF.Relu)

        # ---- z2 = h1 W2^T  [128 (m), 96 (l)]; lives in P cols 0:96 ----
        P = psP.tile([128, 196], FP32, tag="P")
        z2P = P[:, 0:96]
        for c in range(4):
            nc.tensor.matmul(z2P[:], h1t[:, 128 * c:128 * (c + 1)],
                             w2t[:, 96 * c:96 * (c + 1)],
                             start=(c == 0), stop=(c == 3))
        # z2w: wrap-padded [h2[95], h2[0..95], h2[0], h2[1]]
        z2w = sb.tile([128, 99], BF16, tag="z2w")
        nc.scalar.activation(out=z2w[:, 1:97], in_=z2P[:], func=AF.Copy)
        nc.scalar.activation(out=z2w[:, 0:1], in_=z2P[:, 95:96], func=AF.Copy)
        nc.scalar.activation(out=z2w[:, 97:99], in_=z2P[:, 0:2], func=AF.Copy)

        # ---- h2 = A z2 into P [128, (g, 98)] via 4 wrap-wide matmuls ----
        # g1 operands brought to partitions 0:64 (tile_position mixing
        # of (0,64)/(64,64) groups faults the device)
        atl1 = sb.tile([64, 64], BF16, tag="atl1")
        nc.vector.tensor_copy(out=atl1[:], in_=atil[64:128, 64:128])
        z2lo = sb.tile([64, 99], BF16, tag="z2lo")
        nc.vector.tensor_copy(out=z2lo[:], in_=z2w[64:128, :])
        for j in range(2):
            lhs = atil[0:64, 0:64] if j == 0 else atl1[:]
            rhs = z2w[0:64, :] if j == 0 else z2lo[:]
            base = 98 * j
            nc.tensor.matmul(P[0:64, base:base + 98], lhs, rhs[:, 0:98],
                             start=True, stop=True)
            nc.tensor.matmul(P[64:128, base:base + 98], lhs, rhs[:, 1:99],
                             start=True, stop=True, tile_position=(0, 64))
        if pr == 0 and "dbg2" in outs:
            nc.sync.dma_start(out=outs["dbg2"][0:128, 0:99], in_=z2w[:])
        HH = sb.tile([128, 196], BF16, tag="HH")
        if has_b2:
            nc.vector.tensor_tensor(out=HH[:], in0=P[:], in1=b2r[:], op=OP.add)
        else:
            nc.vector.tensor_copy(out=HH[:], in_=P[:])

        if pr == 0 and "dbg2" in outs:
            nc.sync.dma_start(out=outs["dbg2"][0:128, 99:295], in_=HH[:])
        # ---- conv: y[oc, (g, l)] ----
        HH_A = HH[:].rearrange("p (g w) -> p g w", w=98)[:, :, 0:96]
        HH_B = HH[0:64, :].rearrange("p (g w) -> p g w", w=98)[:, :, 2:98]
        yA = psY.tile([128, 384], FP32, tag="yA")
        yB = psY.tile([128, 384], FP32, tag="yB")
        for oc in range(4):
            out_sl = (yA if oc < 2 else yB)[:, 192 * (oc % 2):192 * (oc % 2 + 1)]
            nc.tensor.matmul(out_sl, cwAB[:, 128 * oc:128 * (oc + 1)],
                             HH_A, start=True, stop=False)
            nc.tensor.matmul(out_sl, cwC[:, 128 * oc:128 * (oc + 1)],
                             HH_B, start=False, stop=True)

        # ---- y evac (DVE + Act halves) + one DMA per pair ----
        ysb = sb_y.tile([128, 768], BF16, tag="ysb")
        nc.vector.tensor_copy(out=ysb[:, 0:384], in_=yA[:])
        nc.scalar.activation(out=ysb[:, 384:768], in_=yB[:], func=AF.Copy)
        if pr == 0 and "dbg3" in outs:
            nc.sync.dma_start(out=outs["dbg3"][:], in_=ysb[:])
        dst = y_d[:, :, pr, :].rearrange("oc p j -> p oc j")
        nc.sync.dma_start(out=dst, in_=ysb[:].rearrange(
            "p (oc j) -> p oc j", oc=4))

    ctx.close()


# ---------------- host side ----------------

def _prep_consts(W1, b1, W2, b2, conv_w):
    bf = ml_dtypes.bfloat16
    iota = np.broadcast_to((np.arange(1024) % 128).astype(bf), (128, 1024))
    id128 = np.eye(128).astype(bf)
    ones = np.ones((128, 1), bf)
    cw = conv_w.astype(np.float32)          # [512, 64, 3]
    cw0 = cw[:, :, 0].T                     # [64, 512]
    cw1 = cw[:, :, 1].T
    cw2 = cw[:, :, 2].T
    cwAB = np.concatenate([cw0, cw1], axis=0).astype(bf)       # [128, 512]
    w2t = np.ascontiguousarray(
        W2.T.reshape(4, 128, 96).transpose(1, 0, 2).reshape(128, 384)
    ).astype(bf)
    w1t = np.zeros((128, 512), bf)
    w1t[0:96, :] = np.ascontiguousarray(W1.T).astype(bf)
    cwC = np.zeros((128, 512), bf)
    cwC[0:64, :] = cw2.astype(bf)
    cst = np.concatenate(
        [np.ascontiguousarray(iota), id128, ones, cwAB, w2t, w1t, cwC],
        axis=1)
    consts = dict(cst=np.ascontiguousarray(cst))
    has_b1 = bool(np.any(b1))
    has_b2 = bool(np.any(b2))
    if has_b1:
        consts["b1c"] = np.ascontiguousarray(
            b1.reshape(4, 128).T).astype(np.float32)
    if has_b2:
        pad = np.concatenate([b2[95:96], b2, b2[0:1]])        # [98]
        row = np.tile(pad, 2)                                  # [196]
        consts["b2r"] = np.ascontiguousarray(
            np.broadcast_to(row.astype(bf), (128, 196)))
    return consts, has_b1, has_b2


_NC_CACHE = {}


def _get_nc(has_b1, has_b2):
    key = (has_b1, has_b2)
    if key in _NC_CACHE:
        return _NC_CACHE[key]
    nc = bacc.Bacc("TRN2", target_bir_lowering=False, debug=False)
    Wc = 1024 + 128 + 1 + 512 + 384 + 512 + 512
    ins = {
        "xt": nc.dram_tensor("xt", [128, NPAIR * 96], BF16,
                             kind="ExternalInput").ap(),
        "et": nc.dram_tensor("et", [128, 512], BF16,
                             kind="ExternalInput").ap(),
        "cst": nc.dram_tensor("cst", [128, Wc], BF16,
                              kind="ExternalInput").ap(),
    }
    if has_b1:
        ins["b1c"] = nc.dram_tensor("b1c", [128, 4], FP32,
                                    kind="ExternalInput").ap()
    if has_b2:
        ins["b2r"] = nc.dram_tensor("b2r", [128, 196], BF16,
                                    kind="ExternalInput").ap()
    outs = {
        "y": nc.dram_tensor("y", [4, 128, NPAIR, 192], BF16,
                            kind="ExternalOutput").ap(),
        "dbg": nc.dram_tensor("dbg", [128, 1280], BF16,
                              kind="ExternalOutput").ap(),
        "dbgf": nc.dram_tensor("dbgf", [128, 2], FP32,
                               kind="ExternalOutput").ap(),
        "dbg2": nc.dram_tensor("dbg2", [128, 295], BF16,
                               kind="ExternalOutput").ap(),
        "dbg3": nc.dram_tensor("dbg3", [128, 768], BF16,
                               kind="ExternalOutput").ap(),
    }
    with tile.TileContext(nc) as tc:
        build_gcn_kernel(tc, outs, ins, has_b1, has_b2)
    nc.compile()
    _NC_CACHE[key] = nc
    return nc


def kernel(x, edge_index, W1, b1, W2, b2, conv_w, _trace=False):
    x = np.asarray(x, dtype=np.float32)
    edge_index = np.asarray(edge_index)
    consts, has_b1, has_b2 = _prep_consts(
        np.asarray(W1, np.float32), np.asarray(b1, np.float32),
        np.asarray(W2, np.float32), np.asarray(b2, np.float32),
        np.asarray(conv_w, np.float32))
    nc = _get_nc(has_b1, has_b2)

    bf = ml_dtypes.bfloat16
    in_maps = []
    for core in range(N_CORES):
        sl = slice(core * G, (core + 1) * G)
        xs = x[sl]                                   # [64, 96, 64]
        xt = np.ascontiguousarray(
            xs.transpose(0, 2, 1).reshape(128 * NPAIR, 96)
            .reshape(NPAIR, 128, 96).transpose(1, 0, 2).reshape(128, -1)
        ).astype(bf)
        ei = edge_index[sl].astype(np.int64)          # [64, 2, 512]
        # odd (pair-local g=1) graphs' ids offset by +64 for block one-hots
        ei = ei + 64 * (np.arange(G)[:, None, None] % 2)
        # et[p, (c, g, side)] = ei[g, side, c*128+p]
        et = np.ascontiguousarray(
            ei.reshape(G, 2, 4, 128).transpose(3, 2, 0, 1).reshape(128, 512)
        ).astype(bf)
        m = dict(consts)
        m["xt"] = xt
        m["et"] = et
        in_maps.append(m)

    res = run_bass_kernel_spmd(nc, in_maps, core_ids=list(range(N_CORES)),
                               trace=_trace)
    out = np.empty((B, S, H), np.float32)
    for core in range(N_CORES):
        yT = res.results[core]["y"].astype(np.float32)  # [4, 128, 32, 192]
        yc = yT.reshape(4, 128, NPAIR, 2, 96).transpose(2, 3, 4, 0, 1) \
               .reshape(G, 96, 512)
        out[core * G:(core + 1) * G] = yc
    if _trace:
        kernel.last_results = res
    return out
